# revision 12
# baseline (speedup 1.0000x reference)
"""Trainium2 Bass kernel for CompoundWordAutoregressiveWrapper loss_fn.

Computes 8 scalar losses:
  - 7 masked-mean cross-entropy losses, one per projection head
    ([2,1024,6913] logits each), target channels 0..6 of x[:,1:,:],
    mask = (x[:,1:,0] != 0).
  - 1 masked-mean MSE between a constant f0 (the "temps" branch of the
    reference constant-folds: softmax over an axis of size 1 is
    identically 1.0, so f is input-independent) and x[:,1:,11].

Strategy (data-parallel, per sharding hint): flatten p = B*S = 2048 rows,
shard 256 rows to each of 8 NeuronCores. Each core:
  - streams its 7x[256,6913] logit slices from HBM once, ALL on the
    sync-engine HWDGE ring. Measured: one ring drives all 16 SDMA
    engines at the ~427 GB/s per-core read ceiling (the sync queue does
    nothing else, so pool-recycling waits never block compute). The
    stream is ~116us; a loads-only probe bounds the whole envelope at
    ~129us.
  - ScalarE exps every element (~97us at the worst-case ~1.0 GHz DVFS
    state; the ACT clock varies run-to-run). Early tiles use one
    Exp+accumulator pass per tile; late tiles split [4352, 2561] with
    the second chunk's row-sum done by the otherwise-idle VectorE
    (reduce_sum over bf16) to cut ScalarE's per-tile latency before the
    stream ends; the last tile is [2801 (VectorE), 4112 (accum)]. This
    schedule is chosen by simulating DMA/ACT/DVE clock corners so the
    exposed compute after the last byte stays ~2-5us in the worst case.
  - logits[row, target[row]] is fetched by indirect (gather) DMA from
    DRAM via SWDGE using host-precomputed flat element offsets;
  - one [128, 34] tile (20 sumexp cols + 14 gathered-logit cols) is
    DMA'd out; the O(rows) epilogue (log, masked sums, the input-only
    MSE term, and the cross-core scalar all-reduce) runs on the host
    during unsharding.
"""

import sys

if "/opt/trn_rl_repo" not in sys.path:
    sys.path.insert(0, "/opt/trn_rl_repo")

import numpy as np

_B, _S = 2, 1024
_P = _B * _S  # 2048 flattened rows
_V = 6913
_NCORES = 8
_ROWS = _P // _NCORES  # 256 rows per core
_HEADS = (
    "proj_type",
    "proj_barbeat",
    "proj_tempo",
    "proj_instrument",
    "proj_note_name",
    "proj_octave",
    "proj_duration",
)
_NHEADS = len(_HEADS)
_NTILES = _ROWS // 128  # 2
_NITER = _NTILES * _NHEADS  # 14

# Per-iteration chunk plan: (size, mode) with mode 'A' = ScalarE
# Exp+accumulator, 'D' = ScalarE Exp + VectorE reduce_sum. Chosen by
# corner-case simulation (DMA 1.16-1.2 ns/col, ACT 0.83-1.05 ns/elem,
# DVE 1.06-1.32 ns/elem, DMA-semaphore completion lag 0.5-2.5us).
# Two chunks per tile are mandatory: a single-chunk tile exposes the
# full DMA-completion-semaphore lag before every exp, which at the
# throttled-clock corner feeds back through pool recycling into a
# ~10.4us/tile death spiral (measured).
# One accumulator chunk + one VectorE-reduced chunk per tile. All-DVE
# ([D,D]) tiles were tried to shed accumulator reads but measurably
# stall at the throttled-VectorE corner: with two D-chunks per tile the
# 3-buffer esd ring holds only 1.5 tiles of slack, and DVE trailing ~2
# chunks gates the exp chain (~10.5us/tile cadence collapse).
_PLAN = [((3456, "A"), (3457, "D"))] * 13 + [((3409, "A"), (3504, "A"))]
assert len(_PLAN) == _NITER and all(sum(s for s, _ in p) == _V for p in _PLAN)

# outb columns: [0:14] primary sumexp per iteration (col = t*7+h),
# [14:28] the second chunk of each iteration, [28:42] gathered logits
# (col 28 + t*7+h).
_GCOL = 28
_NOUT = _GCOL + _NITER  # 42

# f = (s @ d)/6 with s identically 6.0 -> f[...,0] = column sum of
# sin(1*ang) over the 6912-entry trig table; mathematically ~0, fp
# residual ~1.6e-5 (impact on the MSE is ~4e-8 relative).
_F0 = 1.6023243915697094e-05

_PROGRAM_CACHE = {}


def _build(rows=_ROWS, v=_V):
    """Build the SPMD Bass program for one core: rows x v per head."""
    import concourse.bass as bass
    import concourse.mybir as mybir
    from concourse import bacc, tile

    f32 = mybir.dt.float32
    i32 = mybir.dt.int32
    bf16 = mybir.dt.bfloat16
    AF = mybir.ActivationFunctionType

    assert rows % 128 == 0
    ntiles = rows // 128

    # Bacc (not plain Bass): its compile() legalizes multi-wait sync via
    # InstEventSemaphore -- TRN2 compute instructions encode at most 1 wait.
    nc = bacc.Bacc(trn_type="TRN2")
    # 1-D logits tensors: the flat view is what the gather DMA indexes into;
    # the streaming loads re-view them as [rows, v].
    lg_dram = [
        nc.dram_tensor(f"lg{h}", [rows * v], f32, kind="ExternalInput")
        for h in range(_NHEADS)
    ]
    # goff[r, h] = r*v + target[r, h]: flat element offsets for the gather
    goff_dram = nc.dram_tensor("goff", [rows, 8], i32, kind="ExternalInput")
    out_dram = nc.dram_tensor("out", [128, _NOUT], f32, kind="ExternalOutput")

    lg2d = [d.rearrange("(r c) -> r c", c=v) for d in lg_dram]
    # [N, 1] view for the gather: offsets index axis 0, one element each
    lgflat = [d.rearrange("(n o) -> n o", o=1) for d in lg_dram]

    wa = max(s for p in _PLAN for s, m in p if m == "A")
    wd = max(s for p in _PLAN for s, m in p if m == "D")

    with tile.TileContext(nc) as tc:
        with (
            tc.tile_pool(name="lg", bufs=6) as lgp,
            tc.tile_pool(name="esa", bufs=1) as esap,
            tc.tile_pool(name="esd", bufs=3) as esdp,
            tc.tile_pool(name="sm", bufs=1) as smp,
        ):
            # small loads on SWDGE so the sync HWDGE ring starts with the
            # big streaming loads
            goff = []
            for t in range(ntiles):
                g = smp.tile([128, 8], i32, tag=f"goff{t}")
                nc.gpsimd.dma_start(g[:], goff_dram[t * 128 : (t + 1) * 128, :])
                goff.append(g)
            outb = smp.tile([128, _NOUT], f32, tag="outb")

            nextra = 0
            for h in range(_NHEADS):
                for t in range(ntiles):
                    i = h * ntiles + t
                    col = t * _NHEADS + h
                    plan = _PLAN[i]
                    lg = lgp.tile([128, v], f32, tag="lg")
                    src = lg2d[h][t * 128 : (t + 1) * 128, :]
                    a = 0
                    for ci, (s, mode) in enumerate(plan):
                        b = a + s
                        nc.sync.dma_start(lg[:, a:b], src[:, a:b])
                        if ci == 0:
                            cc = col
                        else:
                            cc = _NITER + nextra
                            nextra += 1
                        # exp output is scratch, written from column 0 of a
                        # dedicated buffer. 'A' chunks use a single reader-
                        # less buffer (no cross-engine WAW at all); 'D'
                        # chunks use a 3-deep pool so the VectorE read only
                        # gates an exp three D-chunks later -- a 2-deep
                        # shared pool measurably locks the ACT->DVE->DMA
                        # ring into a serial ~10.4us/tile limit cycle.
                        if mode == "A":
                            es = esap.tile([128, wa], bf16, tag="esa")
                            nc.scalar.activation(
                                es[:, 0:s],
                                lg[:, a:b],
                                AF.Exp,
                                accum_out=outb[:, cc : cc + 1],
                            )
                        else:
                            es = esdp.tile([128, wd], bf16, tag="esd")
                            nc.scalar.activation(es[:, 0:s], lg[:, a:b], AF.Exp)
                            nc.vector.reduce_sum(
                                outb[:, cc : cc + 1],
                                es[:, 0:s],
                                axis=mybir.AxisListType.X,
                            )
                        a = b
            assert _NITER + nextra == _GCOL

            # gather DMAs: one per (head, row-tile), indexing DRAM directly;
            # tiny SWDGE traffic fully overlapped with the streaming loads
            for h in range(_NHEADS):
                for t in range(ntiles):
                    gc = _GCOL + t * _NHEADS + h
                    nc.gpsimd.indirect_dma_start(
                        out=outb[:, gc : gc + 1],
                        out_offset=None,
                        in_=lgflat[h][:],
                        in_offset=bass.IndirectOffsetOnAxis(
                            ap=goff[t][:, h : h + 1], axis=0
                        ),
                    )

            nc.sync.dma_start(out_dram[:], outb[:])

    return nc


def _get_program():
    if "nc" not in _PROGRAM_CACHE:
        nc = _build()
        nc.finalize()
        _PROGRAM_CACHE["nc"] = nc
    return _PROGRAM_CACHE["nc"]


def _make_in_maps(inputs):
    heads = [
        np.ascontiguousarray(np.asarray(inputs[n], dtype=np.float32)).reshape(_P * _V)
        for n in _HEADS
    ]
    x = np.asarray(inputs["x"])
    tgt = x[:, 1:, :].reshape(_P, 12)
    goff = np.zeros((_P, 8), np.int32)
    rloc = (np.arange(_P, dtype=np.int64) % _ROWS) * _V
    for h in range(_NHEADS):
        goff[:, h] = (rloc + tgt[:, h].astype(np.int64)).astype(np.int32)
    in_maps = []
    for c in range(_NCORES):
        sl = slice(c * _ROWS, (c + 1) * _ROWS)
        fl = slice(c * _ROWS * _V, (c + 1) * _ROWS * _V)
        m = {f"lg{h}": heads[h][fl] for h in range(_NHEADS)}
        m["goff"] = goff[sl]
        in_maps.append(m)
    return in_maps


def _combine(core_outs, x):
    """core_outs: [ncores, 128, _NOUT] -> [8] float32 losses.

    Host epilogue: masked sums across rows, the input-only MSE term, and
    the cross-core scalar reduction.
    """
    o = np.asarray(core_outs, dtype=np.float64)  # [C, 128, _NOUT]
    sumexp = o[:, :, 0:_NITER].copy()
    # fold each multi-chunk iteration's extra column into its primary col
    nextra = 0
    for i, plan in enumerate(_PLAN):
        t, h = i % _NTILES, i // _NTILES
        col = t * _NHEADS + h
        for _ in plan[1:]:
            sumexp[:, :, col] += o[:, :, _NITER + nextra]
            nextra += 1
    picked = o[:, :, _GCOL : _GCOL + _NITER]
    # [C, 128, t, h] -> flat row r = c*ROWS + t*128 + p
    lse = np.log(sumexp).reshape(_NCORES, 128, _NTILES, _NHEADS)
    pick = picked.reshape(_NCORES, 128, _NTILES, _NHEADS)
    nll = (lse - pick).transpose(0, 2, 1, 3).reshape(_P, _NHEADS)

    tgt = np.asarray(x)[:, 1:, :].reshape(_P, 12)
    mask = (tgt[:, 0] != 0).astype(np.float64)
    tot = mask.sum()
    if tot == 0.0:
        return np.zeros(8, np.float32)
    ce = (nll * mask[:, None]).sum(axis=0) / tot
    t11 = tgt[:, 11].astype(np.float64)
    mse = (mask * (t11 - _F0) ** 2).sum() / tot
    return np.concatenate([ce, [mse]]).astype(np.float32)


def _execute(inputs, trace=False, **kwargs):
    from concourse import bass_utils

    nc = _get_program()
    in_maps = _make_in_maps(inputs)
    res = bass_utils.run_bass_kernel_spmd(
        nc, in_maps, core_ids=list(range(_NCORES)), trace=trace, **kwargs
    )
    core_outs = np.stack([np.asarray(r["out"]) for r in res.results])
    return _combine(core_outs, inputs["x"]), res


def kernel(**inputs) -> np.ndarray:
    out, _ = _execute(inputs)
    return out
